# revision 10
# baseline (speedup 1.0000x reference)
"""Self-attention kernel for Trainium2, 8 NeuronCores, data-parallel over batch.

Reference computation (per batch sample, N=H*W=4096, C=64, Ck=8):
    f = x @ Wf + bf            [N, 8]
    g = x @ Wg + bg            [N, 8]
    h = x @ Wh + bh            [N, 64]
    s = f @ g^T                [N, N]
    attn = softmax(s, axis=-1)
    o = gamma * (attn @ h) + x

Kernel strategy (one sample per core):
  - All matmuls in bf16 on TensorE; scores computed TRANSPOSED: sT[m, n] with
    m (the softmax-reduction index) on partitions.  Softmax then needs no max
    subtraction (scores are O(1)) and the denominator comes for free from an
    augmented ones-column in h: ctxT = [gamma*h | 1]^T @ exp(sT) accumulated in
    PSUM over m-tiles; row 64 of ctxT is the softmax denominator.
  - exp on ScalarE (PSUM -> SBUF bf16).  Scores are pre-scaled by C1 on the f
    side (folded into Wf on host) so a later DVE bit-trick exp can share the
    work; ScalarE undoes the scale via the activation's free affine.
  - Epilogue: PE-transpose ctxT chunks back to [n, c] layout, per-partition
    reciprocal of den, scale + residual add, DMA out.
"""

import numpy as np
import ml_dtypes

import concourse.bass as bass
import concourse.mybir as mybir
import concourse.tile as tile
from concourse.bass import ts, ds
from concourse.bass_utils import run_bass_kernel_spmd

BF16 = mybir.dt.bfloat16
F32 = mybir.dt.float32

N = 4096          # H*W per sample
C = 64            # channels
CK = 8            # f/g projection dim
P = 128           # partitions
NT = N // P       # 32 n/m tiles
HALF = N // 2     # 2048
HT = HALF // P    # 16 tiles per half
C1 = 128.0 * np.log2(np.e)   # score pre-scale (f side), undone by ACT affine

# fraction of each 2048-wide exp chunk done on the DVE via the bf16 bit trick
# (0 -> all on ScalarE).  Tuned after profiling.
DVE_EXP_COLS = 0


def _np_bf16(a):
    return np.ascontiguousarray(a.astype(np.float32).astype(ml_dtypes.bfloat16))


def prepare_weights(Wf, bf, Wg, bg, Wh, bh, gamma):
    """Host-side weight folding. Returns dict of bf16 arrays (dram params)."""
    Wf = np.asarray(Wf, np.float32)
    Wg = np.asarray(Wg, np.float32)
    Wh = np.asarray(Wh, np.float32)
    bf = np.asarray(bf, np.float32)
    bg = np.asarray(bg, np.float32)
    bh = np.asarray(bh, np.float32)
    gamma = float(np.asarray(gamma, np.float32))

    # f-side, scaled by C1, bias as row 64, zero-padded: [128, 32]
    wf_aug = np.zeros((128, 128), np.float32)
    wf_aug[:C, :CK] = C1 * Wf
    wf_aug[C, :CK] = C1 * bf

    # g-side, 4 replicas at column offsets 0/32/64/96 (row-group packing): [128, 128]
    wg_aug = np.zeros((128, 128), np.float32)
    for b in range(4):
        wg_aug[:C, 32 * b: 32 * b + CK] = Wg
        wg_aug[C, 32 * b: 32 * b + CK] = bg

    # h-side with gamma folded + ones column at 64: [128, 128]
    wh_aug = np.zeros((128, 128), np.float32)
    wh_aug[:C, :C] = gamma * Wh
    wh_aug[C, :C] = gamma * bh
    wh_aug[C, C] = 1.0

    ident = np.eye(128, dtype=np.float32)

    return {
        "wf": _np_bf16(wf_aug),
        "wg": _np_bf16(wg_aug),
        "wh": _np_bf16(wh_aug),
        "ident": _np_bf16(ident),
    }


def _spill_excess_waits(nc, limit=1):
    """Walrus rejects HW-queue instructions carrying more than a couple of
    semaphore waits.  Move excess waits onto standalone EventSemaphore
    instructions inserted just before the offender on the same engine
    (cumulative sem-ge waits split across instructions are equivalent)."""
    n_spill = 0
    for bb in nc.main_func.blocks:
        rebuilt = []
        changed = False
        for ins in bb.instructions:
            si = ins.sync_info
            if si is not None and len(si.on_wait) > limit:
                waits = list(si.on_wait)
                for w in waits[limit:]:
                    ev = mybir.InstEventSemaphore(
                        name=f"wspill-{n_spill}", ins=[], outs=[])
                    ev.engine = ins.engine
                    ev.sync_info = mybir.SyncInfo(on_wait=[w], on_update=[])
                    rebuilt.append(ev)
                    n_spill += 1
                ins.sync_info = mybir.SyncInfo(
                    on_wait=waits[:limit], on_update=list(si.on_update))
                changed = True
            rebuilt.append(ins)
        if changed:
            bb.instructions = rebuilt
    return n_spill


def build_bass():
    """Build the per-core Bass graph (SPMD: same graph on all 8 cores)."""
    nc = bass.Bass()

    x_d = nc.declare_dram_parameter("x", [N, C], F32, isOutput=False)
    wf_d = nc.declare_dram_parameter("wf", [128, 128], BF16, isOutput=False)
    wg_d = nc.declare_dram_parameter("wg", [128, 128], BF16, isOutput=False)
    wh_d = nc.declare_dram_parameter("wh", [128, 128], BF16, isOutput=False)
    id_d = nc.declare_dram_parameter("ident", [128, 128], BF16, isOutput=False)
    out_d = nc.declare_dram_parameter("out", [N, C], F32, isOutput=True)

    with tile.TileContext(nc) as tc:
        _build_body(nc, tc, x_d, wf_d, wg_d, wh_d, id_d, out_d)
    _spill_excess_waits(nc)
    return nc


def _build_body(nc, tc, x_d, wf_d, wg_d, wh_d, id_d, out_d):
    from contextlib import ExitStack

    with ExitStack() as ctx:
        consts = ctx.enter_context(tc.tile_pool(name="consts", bufs=1))
        sbuf = ctx.enter_context(tc.tile_pool(name="sbuf", bufs=1))
        exp_pool = ctx.enter_context(tc.tile_pool(name="expp", bufs=3))
        work = ctx.enter_context(tc.tile_pool(name="work", bufs=3))

        # ---- load constants ----
        wf_sb = consts.tile([128, 128], BF16)
        wg_sb = consts.tile([128, 128], BF16)
        wh_sb = consts.tile([128, 128], BF16)
        id_sb = consts.tile([128, 128], BF16)
        nc.sync.dma_start(wf_sb[:], wf_d[:])
        nc.sync.dma_start(wg_sb[:], wg_d[:])
        nc.sync.dma_start(wh_sb[:], wh_d[:])
        nc.sync.dma_start(id_sb[:], id_d[:])

        # ---- load x:  x_sb[p, t, c] = x[128 t + p, c] ----
        x_sb = consts.tile([P, NT, C], F32)
        x3 = x_d.rearrange("(t p) c -> p t c", p=P)
        nc.sync.dma_start(x_sb[:, :16, :], x3[:, :16, :])
        nc.sync.dma_start(x_sb[:, 16:, :], x3[:, 16:, :])

        # bf16 copy of x for transposes
        x_bf = consts.tile([P, NT, C], BF16)
        nc.any.tensor_copy(x_bf[:, :16, :], x_sb[:, :16, :])
        nc.any.tensor_copy(x_bf[:, 16:, :], x_sb[:, 16:, :])

        # ---- xT_aug [128, N] bf16: rows 0..63 = x^T, row 64 = ones, 65.. = 0 ----
        xt_sb = consts.tile([128, N], BF16)
        nc.vector.memset(xt_sb[C:, :], 0.0)
        nc.vector.memset(xt_sb[C: C + 1, :], 1.0)

        with tc.tile_pool(name="pro_ps", bufs=2, space="PSUM") as pro_ps:
            # transpose x tiles in groups of 4 -> one PSUM bank per group
            for grp in range(NT // 4):
                pt = pro_ps.tile([C, 512], BF16, tag="xtr")
                for j in range(4):
                    t = 4 * grp + j
                    nc.tensor.transpose(pt[:, ts(j, P)], x_bf[:, t, :], id_sb[:])
                nc.any.tensor_copy(xt_sb[:C, ds(grp * 512, 512)], pt[:])

            # fS [128, N] bf16 (rows 0..7 = C1*f^T, rest zero)
            f_sb = consts.tile([128, N], BF16)
            g_sb = consts.tile([128, N], BF16)
            for chunk in range(N // 512):
                pf = pro_ps.tile([128, 512], F32, tag="fg")
                nc.tensor.matmul(pf[:], wf_sb[:, :], xt_sb[:, ts(chunk, 512)],
                                 start=True, stop=True)
                nc.any.tensor_copy(f_sb[:, ts(chunk, 512)], pf[:])
            for chunk in range(N // 512):
                pg = pro_ps.tile([128, 512], F32, tag="fg")
                nc.tensor.matmul(pg[:], wg_sb[:, :], xt_sb[:, ts(chunk, 512)],
                                 start=True, stop=True)
                nc.any.tensor_copy(g_sb[:, ts(chunk, 512)], pg[:])

            # h_aug tiles: h_sb[:, m, :] = [gamma*h | 1 | 0pad] for m-tile rows
            h_sb = consts.tile([P, NT, 128], BF16)
            for grp in range(NT // 4):
                ph = pro_ps.tile([128, 512], F32, tag="h")
                for j in range(4):
                    m = 4 * grp + j
                    nc.tensor.matmul(ph[:, ts(j, P)], xt_sb[:, ts(m, P)],
                                     wh_sb[:], start=True, stop=True)
                nc.any.tensor_copy(h_sb[:, ds(4 * grp, 4), :], ph[:])

        # ---- main: scores -> exp -> ctxT accumulate; then epilogue, per half ----
        with tc.tile_pool(name="ps_s", bufs=1, space="PSUM") as ps_s, \
             tc.tile_pool(name="ps_ctx", bufs=1, space="PSUM") as ps_ctx:
            for half in range(2):
                ctx_ps = ps_ctx.tile([128, HALF], F32, tag="ctx")
                for m in range(NT):
                    s_ps = ps_s.tile([128, HALF], F32, tag="s")
                    for j in range(HALF // 512):
                        nc.tensor.matmul(
                            s_ps[:, ts(j, 512)],
                            g_sb[:, ts(m, P)],
                            f_sb[:, ds(half * HALF + j * 512, 512)],
                            start=True, stop=True)
                    e_sb = exp_pool.tile([128, HALF], BF16, tag="e")
                    act_cols = HALF - DVE_EXP_COLS
                    nc.scalar.activation(e_sb[:, :act_cols], s_ps[:, :act_cols],
                                         mybir.ActivationFunctionType.Exp,
                                         bias=0.0, scale=float(1.0 / C1))
                    if DVE_EXP_COLS:
                        _dve_exp(nc, work, e_sb, s_ps, act_cols, DVE_EXP_COLS)
                    for j in range(HALF // 512):
                        nc.tensor.matmul(
                            ctx_ps[:, ts(j, 512)],
                            h_sb[:, m, :],
                            e_sb[:, ts(j, 512)],
                            start=(m == 0), stop=(m == NT - 1))

                # epilogue for this half
                ctxt_sb = work.tile([128, HALF], BF16, tag="ctxt")
                nc.any.tensor_copy(ctxt_sb[:], ctx_ps[:])
                tr_ps = ps_ctx.tile([128, HALF], BF16, tag="ctx")
                for t in range(HT):
                    nc.tensor.transpose(tr_ps[:, ds(t * P, P)],
                                        ctxt_sb[:, ts(t, P)], id_sb[:])
                for t in range(HT):
                    blk = tr_ps[:, ds(t * P, P)]
                    rden = work.tile([P, 1], F32, tag="rden")
                    nc.vector.reciprocal(rden[:], blk[:, C: C + 1])
                    tmp = work.tile([P, C], F32, tag="tmp")
                    nc.scalar.mul(tmp[:], blk[:, :C], rden[:])
                    osb = work.tile([P, C], F32, tag="osb")
                    nc.vector.tensor_add(osb[:], tmp[:],
                                         x_sb[:, half * HT + t, :])
                    nc.sync.dma_start(
                        out_d[ds((half * HT + t) * P, P), :], osb[:])


def _dve_exp(nc, work, e_sb, s_ps, col0, ncols):
    """bf16 bit-trick exp on the DVE for columns [col0, col0+ncols) of the
    score PSUM tile: i16 = round(s' + C2) reinterpreted as bf16 ~= exp(s).
    s' already includes the C1 = 128*log2(e) scale (folded into Wf)."""
    C2 = 127.0 * 128.0 - 7.4 + 0.5  # bias - Schraudolph shift + round-compensation
    i16_view = e_sb.bitcast(mybir.dt.int16)
    nc.vector.tensor_scalar_add(i16_view[:, ds(col0, ncols)],
                                s_ps[:, ds(col0, ncols)], C2)


_CACHE = {}


def _get_nc():
    if "nc" not in _CACHE:
        _CACHE["nc"] = build_bass()
    return _CACHE["nc"]


def kernel(x, Wf, bf, Wg, bg, Wh, bh, gamma):
    x = np.asarray(x, np.float32)
    B = x.shape[0]
    assert x.shape == (B, 64, 64, 64) and B == 8

    w = prepare_weights(Wf, bf, Wg, bg, Wh, bh, gamma)
    nc = _get_nc()
    in_maps = [
        {"x": np.ascontiguousarray(x[i].reshape(N, C)), **w}
        for i in range(B)
    ]
    res = run_bass_kernel_spmd(nc, in_maps, core_ids=list(range(8)))
    out = np.stack([np.asarray(res.results[i]["out"]).reshape(64, 64, 64)
                    for i in range(B)])
    return out.astype(np.float32)


# revision 15
# speedup vs baseline: 1.3291x; 1.3291x over previous
"""Self-attention kernel for Trainium2, 8 NeuronCores, data-parallel over batch.

Reference computation (per batch sample, N=H*W=4096, C=64, Ck=8):
    f = x @ Wf + bf            [N, 8]
    g = x @ Wg + bg            [N, 8]
    h = x @ Wh + bh            [N, 64]
    s = f @ g^T                [N, N]
    attn = softmax(s, axis=-1)
    o = gamma * (attn @ h) + x

Kernel strategy (one sample per core):
  - All matmuls in bf16 on TensorE; scores computed TRANSPOSED: sT[m, n] with
    m (the softmax-reduction index) on partitions.  Softmax then needs no max
    subtraction (scores are O(1)) and the denominator comes for free from an
    augmented ones-column in h: ctxT = [gamma*h | 1]^T @ exp(sT) accumulated in
    PSUM over m-tiles; row 64 of ctxT is the softmax denominator.
  - exp on ScalarE (PSUM -> SBUF bf16).  Scores are pre-scaled by C1 on the f
    side (folded into Wf on host) so a later DVE bit-trick exp can share the
    work; ScalarE undoes the scale via the activation's free affine.
  - Epilogue: PE-transpose ctxT chunks back to [n, c] layout, per-partition
    reciprocal of den, scale + residual add, DMA out.
"""

import numpy as np
import ml_dtypes

import concourse.bass as bass
import concourse.mybir as mybir
import concourse.tile as tile
from concourse.bass import ts, ds
from concourse.bass_utils import run_bass_kernel_spmd

BF16 = mybir.dt.bfloat16
F32 = mybir.dt.float32

N = 4096          # H*W per sample
C = 64            # channels
CK = 8            # f/g projection dim
P = 128           # partitions
NT = N // P       # 32 n/m tiles
HALF = N // 2     # 2048
HT = HALF // P    # 16 tiles per half
C1 = 128.0 * np.log2(np.e)   # score pre-scale (f side), undone by ACT affine

# columns of each exp chunk done on the DVE via the bf16 bit trick
# (0 -> all on ScalarE).  Tuned after profiling.
DVE_EXP_COLS = 512


def _np_bf16(a):
    return np.ascontiguousarray(a.astype(np.float32).astype(ml_dtypes.bfloat16))


def prepare_weights(Wf, bf, Wg, bg, Wh, bh, gamma):
    """Host-side weight folding. Returns dict of bf16 arrays (dram params)."""
    Wf = np.asarray(Wf, np.float32)
    Wg = np.asarray(Wg, np.float32)
    Wh = np.asarray(Wh, np.float32)
    bf = np.asarray(bf, np.float32)
    bg = np.asarray(bg, np.float32)
    bh = np.asarray(bh, np.float32)
    gamma = float(np.asarray(gamma, np.float32))

    # f-side, scaled by C1, bias as row 64, zero-padded: [128, 32]
    wf_aug = np.zeros((128, 128), np.float32)
    wf_aug[:C, :CK] = C1 * Wf
    wf_aug[C, :CK] = C1 * bf

    # g-side, 4 replicas at column offsets 0/32/64/96 (row-group packing): [128, 128]
    wg_aug = np.zeros((128, 128), np.float32)
    for b in range(4):
        wg_aug[:C, 32 * b: 32 * b + CK] = Wg
        wg_aug[C, 32 * b: 32 * b + CK] = bg

    # h-side with gamma folded + ones column at 64: [128, 128]
    wh_aug = np.zeros((128, 128), np.float32)
    wh_aug[:C, :C] = gamma * Wh
    wh_aug[C, :C] = gamma * bh
    wh_aug[C, C] = 1.0

    ident = np.eye(128, dtype=np.float32)

    return {
        "wf": _np_bf16(wf_aug),
        "wg": _np_bf16(wg_aug),
        "wh": _np_bf16(wh_aug),
        "ident": _np_bf16(ident),
    }


def _spill_excess_waits(nc, limit=1):
    """Walrus rejects HW-queue instructions carrying more than a couple of
    semaphore waits.  Move excess waits onto standalone EventSemaphore
    instructions inserted just before the offender on the same engine
    (cumulative sem-ge waits split across instructions are equivalent)."""
    n_spill = 0
    for bb in nc.main_func.blocks:
        rebuilt = []
        changed = False
        for ins in bb.instructions:
            si = ins.sync_info
            if si is not None and len(si.on_wait) > limit:
                waits = list(si.on_wait)
                for w in waits[limit:]:
                    ev = mybir.InstEventSemaphore(
                        name=f"wspill-{n_spill}", ins=[], outs=[])
                    ev.engine = ins.engine
                    ev.sync_info = mybir.SyncInfo(on_wait=[w], on_update=[])
                    rebuilt.append(ev)
                    n_spill += 1
                ins.sync_info = mybir.SyncInfo(
                    on_wait=waits[:limit], on_update=list(si.on_update))
                changed = True
            rebuilt.append(ins)
        if changed:
            bb.instructions = rebuilt
    return n_spill


def build_bass():
    """Build the per-core Bass graph (SPMD: same graph on all 8 cores)."""
    nc = bass.Bass()

    x_d = nc.declare_dram_parameter("x", [N, C], F32, isOutput=False)
    wf_d = nc.declare_dram_parameter("wf", [128, 128], BF16, isOutput=False)
    wg_d = nc.declare_dram_parameter("wg", [128, 128], BF16, isOutput=False)
    wh_d = nc.declare_dram_parameter("wh", [128, 128], BF16, isOutput=False)
    id_d = nc.declare_dram_parameter("ident", [128, 128], BF16, isOutput=False)
    out_d = nc.declare_dram_parameter("out", [N, C], F32, isOutput=True)

    with tile.TileContext(nc) as tc:
        _build_body(nc, tc, x_d, wf_d, wg_d, wh_d, id_d, out_d)
    _spill_excess_waits(nc)
    return nc


def _build_body(nc, tc, x_d, wf_d, wg_d, wh_d, id_d, out_d):
    from contextlib import ExitStack

    with ExitStack() as ctx:
        consts = ctx.enter_context(tc.tile_pool(name="consts", bufs=1))
        sbuf = ctx.enter_context(tc.tile_pool(name="sbuf", bufs=1))
        exp_pool = ctx.enter_context(tc.tile_pool(name="expp", bufs=3))
        work = ctx.enter_context(tc.tile_pool(name="work", bufs=3))

        # ---- load constants ----
        wf_sb = consts.tile([128, 128], BF16)
        wg_sb = consts.tile([128, 128], BF16)
        wh_sb = consts.tile([128, 128], BF16)
        id_sb = consts.tile([128, 128], BF16)
        nc.sync.dma_start(wf_sb[:], wf_d[:])
        nc.sync.dma_start(wg_sb[:], wg_d[:])
        nc.sync.dma_start(wh_sb[:], wh_d[:])
        nc.sync.dma_start(id_sb[:], id_d[:])

        # ---- load x:  x_sb[p, t, c] = x[128 t + p, c] ----
        x_sb = consts.tile([P, NT, C], F32)
        x3 = x_d.rearrange("(t p) c -> p t c", p=P)
        nc.sync.dma_start(x_sb[:, :16, :], x3[:, :16, :])
        nc.sync.dma_start(x_sb[:, 16:, :], x3[:, 16:, :])

        # bf16 copy of x for transposes
        x_bf = consts.tile([P, NT, C], BF16)
        nc.any.tensor_copy(x_bf[:, :16, :], x_sb[:, :16, :])
        nc.any.tensor_copy(x_bf[:, 16:, :], x_sb[:, 16:, :])

        # ---- xT_aug [128, N] bf16: rows 0..63 = x^T, row 64 = ones, 65.. = 0 ----
        xt_sb = consts.tile([128, N], BF16)
        nc.vector.memset(xt_sb[C:, :], 0.0)
        nc.vector.memset(xt_sb[C: C + 1, :], 1.0)

        with tc.tile_pool(name="pro_ps", bufs=2, space="PSUM") as pro_ps:
            # transpose x tiles in groups of 4 -> one PSUM bank per group
            for grp in range(NT // 4):
                pt = pro_ps.tile([C, 512], BF16, tag="xtr")
                for j in range(4):
                    t = 4 * grp + j
                    nc.tensor.transpose(pt[:, ts(j, P)], x_bf[:, t, :], id_sb[:])
                nc.any.tensor_copy(xt_sb[:C, ds(grp * 512, 512)], pt[:])

            # fS [128, N] bf16 (rows 0..7 = C1*f^T, rest zero)
            f_sb = consts.tile([128, N], BF16)
            g_sb = consts.tile([128, N], BF16)
            for chunk in range(N // 512):
                pf = pro_ps.tile([128, 512], F32, tag="fg")
                nc.tensor.matmul(pf[:], wf_sb[:, :], xt_sb[:, ts(chunk, 512)],
                                 start=True, stop=True)
                nc.any.tensor_copy(f_sb[:, ts(chunk, 512)], pf[:])
            for chunk in range(N // 512):
                pg = pro_ps.tile([128, 512], F32, tag="fg")
                nc.tensor.matmul(pg[:], wg_sb[:, :], xt_sb[:, ts(chunk, 512)],
                                 start=True, stop=True)
                nc.any.tensor_copy(g_sb[:, ts(chunk, 512)], pg[:])

            # h_aug tiles: h_sb[:, m, :] = [gamma*h | 1 | 0pad] for m-tile rows
            h_sb = consts.tile([P, NT, 128], BF16)
            for grp in range(NT // 4):
                ph = pro_ps.tile([128, 512], F32, tag="h")
                for j in range(4):
                    m = 4 * grp + j
                    nc.tensor.matmul(ph[:, ts(j, P)], xt_sb[:, ts(m, P)],
                                     wh_sb[:], start=True, stop=True)
                nc.any.tensor_copy(h_sb[:, ds(4 * grp, 4), :], ph[:])

        # ---- main: scores -> exp -> ctxT accumulate; epilogue, per n-quarter ----
        QW = 1024                      # quarter width
        NQ = N // QW                   # 4
        QT = QW // P                   # n-tiles per quarter
        with tc.tile_pool(name="ps_s", bufs=3, space="PSUM") as ps_s, \
             tc.tile_pool(name="ps_ctx", bufs=1, space="PSUM") as ps_ctx:
            for q in range(NQ):
                ctx_ps = ps_ctx.tile([128, QW], F32, tag="ctx")
                for m in range(NT):
                    s_ps = ps_s.tile([128, QW], F32, tag="s")
                    for j in range(QW // 512):
                        nc.tensor.matmul(
                            s_ps[:, ts(j, 512)],
                            g_sb[:, ts(m, P)],
                            f_sb[:, ds(q * QW + j * 512, 512)],
                            start=True, stop=True)
                    e_sb = exp_pool.tile([128, QW], BF16, tag="e")
                    if (q * NT + m) % 2 == 0:
                        nc.scalar.activation(e_sb[:], s_ps[:],
                                             mybir.ActivationFunctionType.Exp,
                                             bias=0.0, scale=float(1.0 / C1))
                    else:
                        _dve_exp(nc, work, e_sb, s_ps, 0, QW)
                    for j in range(QW // 512):
                        nc.tensor.matmul(
                            ctx_ps[:, ts(j, 512)],
                            h_sb[:, m, :],
                            e_sb[:, ts(j, 512)],
                            start=(m == 0), stop=(m == NT - 1))

                # epilogue for this quarter: DMA-transpose ctxT back to [n, c]
                ctxt_sb = work.tile([128, QW], BF16, tag="ctxt")
                nc.any.tensor_copy(ctxt_sb[:], ctx_ps[:])
                o_tr = work.tile([128, QT, P], BF16, tag="otr")
                for t in range(QT):
                    nc.sync.dma_start_transpose(o_tr[:, t, :],
                                                ctxt_sb[:, ts(t, P)])
                for t in range(QT):
                    blk = o_tr[:, t, :]
                    rden = work.tile([P, 1], F32, tag="rden")
                    nc.vector.reciprocal(rden[:], blk[:, C: C + 1])
                    tmp = work.tile([P, C], F32, tag="tmp")
                    nc.scalar.mul(tmp[:], blk[:, :C], rden[:])
                    osb = work.tile([P, C], F32, tag="osb")
                    nc.vector.tensor_add(osb[:], tmp[:],
                                         x_sb[:, q * QT + t, :])
                    nc.sync.dma_start(
                        out_d[ds((q * QT + t) * P, P), :], osb[:])


def _dve_exp(nc, work, e_sb, s_ps, col0, ncols):
    """bf16 bit-trick exp on the DVE for columns [col0, col0+ncols) of the
    score PSUM tile: i16 = round(s' + C2) reinterpreted as bf16 ~= exp(s).
    s' already includes the C1 = 128*log2(e) scale (folded into Wf)."""
    C2 = 127.0 * 128.0 - 7.5  # bf16 exponent bias - Schraudolph shift
    i16_view = e_sb.bitcast(mybir.dt.int16)
    nc.vector.tensor_scalar_add(i16_view[:, ds(col0, ncols)],
                                s_ps[:, ds(col0, ncols)], C2)


_CACHE = {}


def _get_nc():
    if "nc" not in _CACHE:
        _CACHE["nc"] = build_bass()
    return _CACHE["nc"]


def kernel(x, Wf, bf, Wg, bg, Wh, bh, gamma):
    x = np.asarray(x, np.float32)
    B = x.shape[0]
    assert x.shape == (B, 64, 64, 64) and B == 8

    w = prepare_weights(Wf, bf, Wg, bg, Wh, bh, gamma)
    nc = _get_nc()
    in_maps = [
        {"x": np.ascontiguousarray(x[i].reshape(N, C)), **w}
        for i in range(B)
    ]
    res = run_bass_kernel_spmd(nc, in_maps, core_ids=list(range(8)))
    out = np.stack([np.asarray(res.results[i]["out"]).reshape(64, 64, 64)
                    for i in range(B)])
    return out.astype(np.float32)


# revision 18
# speedup vs baseline: 1.3417x; 1.0096x over previous
"""Self-attention kernel for Trainium2, 8 NeuronCores, data-parallel over batch.

Reference computation (per batch sample, N=H*W=4096, C=64, Ck=8):
    f = x @ Wf + bf            [N, 8]
    g = x @ Wg + bg            [N, 8]
    h = x @ Wh + bh            [N, 64]
    s = f @ g^T                [N, N]
    attn = softmax(s, axis=-1)
    o = gamma * (attn @ h) + x

Kernel strategy (one sample per core):
  - All matmuls in bf16 on TensorE; scores computed TRANSPOSED: sT[m, n] with
    m (the softmax-reduction index) on partitions.  Softmax then needs no max
    subtraction (scores are O(1)) and the denominator comes for free from an
    augmented ones-column in h: ctxT = [gamma*h | 1]^T @ exp(sT) accumulated in
    PSUM over m-tiles; row 64 of ctxT is the softmax denominator.
  - exp on ScalarE (PSUM -> SBUF bf16).  Scores are pre-scaled by C1 on the f
    side (folded into Wf on host) so a later DVE bit-trick exp can share the
    work; ScalarE undoes the scale via the activation's free affine.
  - Epilogue: PE-transpose ctxT chunks back to [n, c] layout, per-partition
    reciprocal of den, scale + residual add, DMA out.
"""

import numpy as np
import ml_dtypes

import concourse.bass as bass
import concourse.mybir as mybir
import concourse.tile as tile
from concourse.bass import ts, ds
from concourse.bass_utils import run_bass_kernel_spmd

BF16 = mybir.dt.bfloat16
F32 = mybir.dt.float32

N = 4096          # H*W per sample
C = 64            # channels
CK = 8            # f/g projection dim
P = 128           # partitions
NT = N // P       # 32 n/m tiles
HALF = N // 2     # 2048
HT = HALF // P    # 16 tiles per half
C1 = 128.0 * np.log2(np.e)   # score pre-scale (f side), undone by ACT affine

# columns of each exp chunk done on the DVE via the bf16 bit trick
# (0 -> all on ScalarE).  Tuned after profiling.
DVE_EXP_COLS = 512


def _np_bf16(a):
    return np.ascontiguousarray(a.astype(np.float32).astype(ml_dtypes.bfloat16))


def prepare_weights(Wf, bf, Wg, bg, Wh, bh, gamma):
    """Host-side weight folding. Returns dict of bf16 arrays (dram params)."""
    Wf = np.asarray(Wf, np.float32)
    Wg = np.asarray(Wg, np.float32)
    Wh = np.asarray(Wh, np.float32)
    bf = np.asarray(bf, np.float32)
    bg = np.asarray(bg, np.float32)
    bh = np.asarray(bh, np.float32)
    gamma = float(np.asarray(gamma, np.float32))

    # f-side, scaled by C1, bias as row 64; 4 replicas at column offsets
    # 0/32/64/96 so row-group-packed matmuls can read fS from any 32-row band
    wf_aug = np.zeros((128, 128), np.float32)
    for b in range(4):
        wf_aug[:C, 32 * b: 32 * b + CK] = C1 * Wf
        wf_aug[C, 32 * b: 32 * b + CK] = C1 * bf

    # g-side, 4 replicas at column offsets 0/32/64/96 (row-group packing): [128, 128]
    wg_aug = np.zeros((128, 128), np.float32)
    for b in range(4):
        wg_aug[:C, 32 * b: 32 * b + CK] = Wg
        wg_aug[C, 32 * b: 32 * b + CK] = bg

    # h-side with gamma folded + ones column at 64: [128, 128]
    wh_aug = np.zeros((128, 128), np.float32)
    wh_aug[:C, :C] = gamma * Wh
    wh_aug[C, :C] = gamma * bh
    wh_aug[C, C] = 1.0

    ident = np.eye(128, dtype=np.float32)

    return {
        "wf": _np_bf16(wf_aug),
        "wg": _np_bf16(wg_aug),
        "wh": _np_bf16(wh_aug),
        "ident": _np_bf16(ident),
    }


def _spill_excess_waits(nc, limit=1):
    """Walrus rejects HW-queue instructions carrying more than a couple of
    semaphore waits.  Move excess waits onto standalone EventSemaphore
    instructions inserted just before the offender on the same engine
    (cumulative sem-ge waits split across instructions are equivalent)."""
    n_spill = 0
    for bb in nc.main_func.blocks:
        rebuilt = []
        changed = False
        for ins in bb.instructions:
            si = ins.sync_info
            if si is not None and len(si.on_wait) > limit:
                waits = list(si.on_wait)
                for w in waits[limit:]:
                    ev = mybir.InstEventSemaphore(
                        name=f"wspill-{n_spill}", ins=[], outs=[])
                    ev.engine = ins.engine
                    ev.sync_info = mybir.SyncInfo(on_wait=[w], on_update=[])
                    rebuilt.append(ev)
                    n_spill += 1
                ins.sync_info = mybir.SyncInfo(
                    on_wait=waits[:limit], on_update=list(si.on_update))
                changed = True
            rebuilt.append(ins)
        if changed:
            bb.instructions = rebuilt
    return n_spill


def build_bass():
    """Build the per-core Bass graph (SPMD: same graph on all 8 cores)."""
    nc = bass.Bass()

    x_d = nc.declare_dram_parameter("x", [N, C], F32, isOutput=False)
    wf_d = nc.declare_dram_parameter("wf", [128, 128], BF16, isOutput=False)
    wg_d = nc.declare_dram_parameter("wg", [128, 128], BF16, isOutput=False)
    wh_d = nc.declare_dram_parameter("wh", [128, 128], BF16, isOutput=False)
    id_d = nc.declare_dram_parameter("ident", [128, 128], BF16, isOutput=False)
    out_d = nc.declare_dram_parameter("out", [N, C], F32, isOutput=True)

    with tile.TileContext(nc) as tc:
        _build_body(nc, tc, x_d, wf_d, wg_d, wh_d, id_d, out_d)
    _spill_excess_waits(nc)
    return nc


def _build_body(nc, tc, x_d, wf_d, wg_d, wh_d, id_d, out_d):
    from contextlib import ExitStack

    with ExitStack() as ctx:
        consts = ctx.enter_context(tc.tile_pool(name="consts", bufs=1))
        sbuf = ctx.enter_context(tc.tile_pool(name="sbuf", bufs=1))
        exp_pool = ctx.enter_context(tc.tile_pool(name="expp", bufs=3))
        work = ctx.enter_context(tc.tile_pool(name="work", bufs=3))

        # ---- load constants ----
        wf_sb = consts.tile([128, 128], BF16)
        wg_sb = consts.tile([128, 128], BF16)
        wh_sb = consts.tile([128, 128], BF16)
        id_sb = consts.tile([128, 128], BF16)
        nc.sync.dma_start(wf_sb[:], wf_d[:])
        nc.sync.dma_start(wg_sb[:], wg_d[:])
        nc.sync.dma_start(wh_sb[:], wh_d[:])
        nc.sync.dma_start(id_sb[:], id_d[:])

        # ---- load x:  x_sb[p, t, c] = x[128 t + p, c] ----
        x_sb = consts.tile([P, NT, C], F32)
        x3 = x_d.rearrange("(t p) c -> p t c", p=P)
        nc.sync.dma_start(x_sb[:, :16, :], x3[:, :16, :])
        nc.sync.dma_start(x_sb[:, 16:, :], x3[:, 16:, :])

        # bf16 copy of x for transposes
        x_bf = consts.tile([P, NT, C], BF16)
        nc.any.tensor_copy(x_bf[:, :16, :], x_sb[:, :16, :])
        nc.any.tensor_copy(x_bf[:, 16:, :], x_sb[:, 16:, :])

        # ---- xT_aug [128, N] bf16: rows 0..63 = x^T, row 64 = ones, 65.. = 0 ----
        xt_sb = consts.tile([128, N], BF16)
        nc.vector.memset(xt_sb[C:, :], 0.0)
        nc.vector.memset(xt_sb[C: C + 1, :], 1.0)

        with tc.tile_pool(name="pro_ps", bufs=2, space="PSUM") as pro_ps:
            # transpose x tiles in groups of 4 -> one PSUM bank per group
            for grp in range(NT // 4):
                pt = pro_ps.tile([C, 512], BF16, tag="xtr")
                for j in range(4):
                    t = 4 * grp + j
                    nc.tensor.transpose(pt[:, ts(j, P)], x_bf[:, t, :], id_sb[:])
                nc.any.tensor_copy(xt_sb[:C, ds(grp * 512, 512)], pt[:])

            # fS [128, N] bf16 (rows 0..7 = C1*f^T, rest zero)
            f_sb = consts.tile([128, N], BF16)
            g_sb = consts.tile([128, N], BF16)
            for chunk in range(N // 512):
                pf = pro_ps.tile([128, 512], F32, tag="fg")
                nc.tensor.matmul(pf[:], wf_sb[:, :], xt_sb[:, ts(chunk, 512)],
                                 start=True, stop=True)
                nc.any.tensor_copy(f_sb[:, ts(chunk, 512)], pf[:])
            for chunk in range(N // 512):
                pg = pro_ps.tile([128, 512], F32, tag="fg")
                nc.tensor.matmul(pg[:], wg_sb[:, :], xt_sb[:, ts(chunk, 512)],
                                 start=True, stop=True)
                nc.any.tensor_copy(g_sb[:, ts(chunk, 512)], pg[:])

            # h_aug tiles: h_sb[:, m, :] = [gamma*h | 1 | 0pad] for m-tile rows
            h_sb = consts.tile([P, NT, 128], BF16)
            for grp in range(NT // 4):
                ph = pro_ps.tile([128, 512], F32, tag="h")
                for j in range(4):
                    m = 4 * grp + j
                    nc.tensor.matmul(ph[:, ts(j, P)], xt_sb[:, ts(m, P)],
                                     wh_sb[:], start=True, stop=True)
                nc.any.tensor_copy(h_sb[:, ds(4 * grp, 4), :], ph[:])

        # ---- main: scores -> exp -> ctxT accumulate; epilogue, per n-quarter ----
        QW = 1024                      # quarter width
        NQ = N // QW                   # 4
        QT = QW // P                   # n-tiles per quarter
        with tc.tile_pool(name="ps_s", bufs=3, space="PSUM") as ps_s, \
             tc.tile_pool(name="ps_ctx", bufs=1, space="PSUM") as ps_ctx:
            # exp engine assignment: ACT chunk ~997ns vs DVE ~1192ns (+ DVE's
            # prologue/epilogue load) -> give ACT ~81 of 128 chunks, spread
            # evenly (Bresenham).
            N_CHUNKS = NQ * NT
            DVE_SHARE = 47
            use_dve = [((i * DVE_SHARE) % N_CHUNKS) < DVE_SHARE
                       for i in range(N_CHUNKS)]

            for q in range(NQ):
                ctx_ps = ps_ctx.tile([128, QW], F32, tag="ctx")
                for mg in range(NT // 2):
                    # two m-tiles' score matmuls packed in 32-row PE tiles
                    sp = [ps_s.tile([128, QW], F32, tag="s", name=f"s{b}")
                          for b in range(2)]
                    for b in range(2):
                        m = 2 * mg + b
                        for j in range(QW // 512):
                            nc.tensor.matmul(
                                sp[b][:, ts(j, 512)],
                                g_sb[ds(32 * b, 32), ts(m, P)],
                                f_sb[ds(32 * b, 32),
                                     ds(q * QW + j * 512, 512)],
                                start=True, stop=True,
                                tile_position=(32 * b, 0))
                    for b in range(2):
                        m = 2 * mg + b
                        e_sb = exp_pool.tile([128, QW], BF16, tag="e")
                        if use_dve[q * NT + m]:
                            _dve_exp(nc, work, e_sb, sp[b], 0, QW)
                        else:
                            nc.scalar.activation(
                                e_sb[:], sp[b][:],
                                mybir.ActivationFunctionType.Exp,
                                bias=0.0, scale=float(1.0 / C1))
                        for j in range(QW // 512):
                            nc.tensor.matmul(
                                ctx_ps[:, ts(j, 512)],
                                h_sb[:, m, :],
                                e_sb[:, ts(j, 512)],
                                start=(m == 0), stop=(m == NT - 1))

                # epilogue for this quarter: DMA-transpose ctxT back to [n, c]
                ctxt_sb = work.tile([128, QW], BF16, tag="ctxt")
                nc.any.tensor_copy(ctxt_sb[:], ctx_ps[:])
                o_tr = work.tile([128, QT, P], BF16, tag="otr")
                for t in range(QT):
                    nc.sync.dma_start_transpose(o_tr[:, t, :],
                                                ctxt_sb[:, ts(t, P)])
                for t in range(QT):
                    blk = o_tr[:, t, :]
                    rden = work.tile([P, 1], F32, tag="rden")
                    nc.vector.reciprocal(rden[:], blk[:, C: C + 1])
                    tmp = work.tile([P, C], F32, tag="tmp")
                    nc.scalar.mul(tmp[:], blk[:, :C], rden[:])
                    osb = work.tile([P, C], F32, tag="osb")
                    nc.vector.tensor_add(osb[:], tmp[:],
                                         x_sb[:, q * QT + t, :])
                    nc.sync.dma_start(
                        out_d[ds((q * QT + t) * P, P), :], osb[:])


def _dve_exp(nc, work, e_sb, s_ps, col0, ncols):
    """bf16 bit-trick exp on the DVE for columns [col0, col0+ncols) of the
    score PSUM tile: i16 = round(s' + C2) reinterpreted as bf16 ~= exp(s).
    s' already includes the C1 = 128*log2(e) scale (folded into Wf)."""
    C2 = 127.0 * 128.0 - 7.5  # bf16 exponent bias - Schraudolph shift
    i16_view = e_sb.bitcast(mybir.dt.int16)
    nc.vector.tensor_scalar_add(i16_view[:, ds(col0, ncols)],
                                s_ps[:, ds(col0, ncols)], C2)


_CACHE = {}


def _get_nc():
    if "nc" not in _CACHE:
        _CACHE["nc"] = build_bass()
    return _CACHE["nc"]


def kernel(x, Wf, bf, Wg, bg, Wh, bh, gamma):
    x = np.asarray(x, np.float32)
    B = x.shape[0]
    assert x.shape == (B, 64, 64, 64) and B == 8

    w = prepare_weights(Wf, bf, Wg, bg, Wh, bh, gamma)
    nc = _get_nc()
    in_maps = [
        {"x": np.ascontiguousarray(x[i].reshape(N, C)), **w}
        for i in range(B)
    ]
    res = run_bass_kernel_spmd(nc, in_maps, core_ids=list(range(8)))
    out = np.stack([np.asarray(res.results[i]["out"]).reshape(64, 64, 64)
                    for i in range(B)])
    return out.astype(np.float32)


# revision 44
# speedup vs baseline: 2.8754x; 2.1431x over previous
"""Self-attention kernel for Trainium2, 8 NeuronCores, data-parallel over batch.

Reference computation (per batch sample, N=H*W=4096, C=64, Ck=8):
    f = x @ Wf + bf            [N, 8]
    g = x @ Wg + bg            [N, 8]
    h = x @ Wh + bh            [N, 64]
    s = f @ g^T                [N, N]
    attn = softmax(s, axis=-1)
    o = gamma * (attn @ h) + x

Kernel strategy (one sample per core):
  - Scores computed TRANSPOSED: sT[m, n] with m (the softmax-reduction index)
    on partitions, via K=9 bf16 matmuls packed two-at-a-time into 32-row
    PE tile_position row groups.  No max subtraction (scores are O(1)); the
    softmax denominator comes free from an augmented column in h.
  - exp split across ScalarE (true exp via activation affine) and VectorE
    (fp8e4m3 Schraudolph bit-trick: i8 = max(s'/16, 0) bitcast to e4m3
    = exp(s)/8), alternating whole [128, 1024] chunks.  Scores carry a
    C1=128*log2(e) scale and +504 offset folded into the weights.
  - ctx^T = [128*gamma*h | 128]^T @ exp accumulated in PSUM over m with
    fp8 DoubleRow matmuls (two m-tiles per instruction); row 64 gives
    128*sum(exp), whose reciprocal directly yields gamma*ctx.
  - Epilogue: DMA-transpose ctxT back to [n, c] layout, per-partition
    reciprocal, scale on DVE, residual add on GpSimd, DMA out.
"""

import numpy as np
import ml_dtypes

import concourse.bass as bass
import concourse.mybir as mybir
import concourse.tile as tile
from concourse.bass import ts, ds
from concourse.bass_utils import run_bass_kernel_spmd

BF16 = mybir.dt.bfloat16
FP8 = mybir.dt.float8e4
F32 = mybir.dt.float32

N = 4096          # H*W per sample
C = 64            # channels
CK = 8            # f/g projection dim
P = 128           # partitions
NT = N // P       # 32 n/m tiles
HALF = N // 2     # 2048
HT = HALF // P    # 16 tiles per half
C1 = 128.0 * np.log2(np.e)   # score pre-scale (f side), undone by ACT affine
SCORE_OFF = 504.0            # additive score offset (exact in bf16):
                             # s'/16 = 8*log2(e)*s + 31.5, the e4m3 bit value
                             # of exp(s)/8 with the Schraudolph shift; clamped
                             # at 0 on the DVE.  ACT computes exp(s - ln 8).
                             # The /8 keeps exp in e4m3 range; softmax ratios
                             # are unaffected.

def _np_bf16(a):
    return np.ascontiguousarray(a.astype(np.float32).astype(ml_dtypes.bfloat16))


def prepare_weights(Wf, bf, Wg, bg, Wh, bh, gamma):
    """Host-side weight folding. Returns dict of bf16 arrays (dram params)."""
    Wf = np.asarray(Wf, np.float32)
    Wg = np.asarray(Wg, np.float32)
    Wh = np.asarray(Wh, np.float32)
    bf = np.asarray(bf, np.float32)
    bg = np.asarray(bg, np.float32)
    bh = np.asarray(bh, np.float32)
    gamma = float(np.asarray(gamma, np.float32))

    # f-side, scaled by C1, bias as row 64; replicated across the 4 32-row
    # bands so row-group-packed score matmuls can read from any band.
    # Column 8 (paired with g-side column 8 == 1) adds SCORE_OFF to every
    # score so the DVE fp8 bit-trick can clamp at 0 instead of going
    # negative: raw scores' = C1*s + SCORE_OFF.
    wf_aug = np.zeros((128, 128), np.float32)
    for b in range(4):
        wf_aug[:C, 32 * b: 32 * b + CK] = C1 * Wf
        wf_aug[C, 32 * b: 32 * b + CK] = C1 * bf
        wf_aug[C, 32 * b + CK] = SCORE_OFF

    # g-side, same replication, column 8 = ones row
    wg_aug = np.zeros((128, 128), np.float32)
    for b in range(4):
        wg_aug[:C, 32 * b: 32 * b + CK] = Wg
        wg_aug[C, 32 * b: 32 * b + CK] = bg
        wg_aug[C, 32 * b + CK] = 1.0

    # h-side scaled by 128*gamma (keeps fp8 h out of subnormals) and a
    # 128-valued denominator column at 64; the epilogue's reciprocal of
    # 128*sum(exp) then yields gamma*ctx directly: [128, 128]
    wh_aug = np.zeros((128, 128), np.float32)
    wh_aug[:C, :C] = 128.0 * gamma * Wh
    wh_aug[C, :C] = 128.0 * gamma * bh
    wh_aug[C, C] = 128.0

    ident = np.eye(128, dtype=np.float32)

    return {
        "wf": _np_bf16(wf_aug),
        "wg": _np_bf16(wg_aug),
        "wh": _np_bf16(wh_aug),
        "ident": _np_bf16(ident),
    }


def _spill_excess_waits(nc, limit=1):
    """Walrus rejects HW-queue instructions carrying more than a couple of
    semaphore waits.  Move excess waits onto standalone EventSemaphore
    instructions inserted just before the offender on the same engine
    (cumulative sem-ge waits split across instructions are equivalent)."""
    n_spill = 0
    for bb in nc.main_func.blocks:
        rebuilt = []
        changed = False
        for ins in bb.instructions:
            si = ins.sync_info
            if si is not None and len(si.on_wait) > limit:
                waits = list(si.on_wait)
                for w in waits[limit:]:
                    ev = mybir.InstEventSemaphore(
                        name=f"wspill-{n_spill}", ins=[], outs=[])
                    ev.engine = ins.engine
                    ev.sync_info = mybir.SyncInfo(on_wait=[w], on_update=[])
                    rebuilt.append(ev)
                    n_spill += 1
                ins.sync_info = mybir.SyncInfo(
                    on_wait=waits[:limit], on_update=list(si.on_update))
                changed = True
            rebuilt.append(ins)
        if changed:
            bb.instructions = rebuilt
    return n_spill


def build_bass(repeat=1, spill=True):
    """Build the per-core Bass graph (SPMD: same graph on all 8 cores).
    repeat > 1 duplicates the whole body for timing calibration."""
    nc = bass.Bass()

    x_d = nc.declare_dram_parameter("x", [N, C], F32, isOutput=False)
    wf_d = nc.declare_dram_parameter("wf", [128, 128], BF16, isOutput=False)
    wg_d = nc.declare_dram_parameter("wg", [128, 128], BF16, isOutput=False)
    wh_d = nc.declare_dram_parameter("wh", [128, 128], BF16, isOutput=False)
    id_d = nc.declare_dram_parameter("ident", [128, 128], BF16, isOutput=False)
    out_d = nc.declare_dram_parameter("out", [N, C], F32, isOutput=True)

    with tile.TileContext(nc) as tc:
        for _ in range(repeat):
            _build_body(nc, tc, x_d, wf_d, wg_d, wh_d, id_d, out_d)
    if spill:
        _spill_excess_waits(nc)
    return nc


def _build_body(nc, tc, x_d, wf_d, wg_d, wh_d, id_d, out_d):
    from contextlib import ExitStack

    with ExitStack() as ctx:
        consts = ctx.enter_context(tc.tile_pool(name="consts", bufs=1))
        sbuf = ctx.enter_context(tc.tile_pool(name="sbuf", bufs=1))
        exp_pool = ctx.enter_context(tc.tile_pool(name="expp", bufs=4))
        work = ctx.enter_context(tc.tile_pool(name="work", bufs=6))

        # ---- load constants ----
        wf_sb = consts.tile([128, 128], BF16)
        wg_sb = consts.tile([128, 128], BF16)
        wh_sb = consts.tile([128, 128], BF16)
        id_sb = consts.tile([128, 128], BF16)
        nc.sync.dma_start(wf_sb[:], wf_d[:])
        nc.sync.dma_start(wg_sb[:], wg_d[:])
        nc.sync.dma_start(wh_sb[:], wh_d[:])
        nc.sync.dma_start(id_sb[:], id_d[:])

        # ---- load x:  x_sb[p, t, c] = x[128 t + p, c] ----
        x_sb = consts.tile([P, NT, C], F32)
        x3 = x_d.rearrange("(t p) c -> p t c", p=P)
        for d in range(8):
            nc.sync.dma_start(x_sb[:, ds(4 * d, 4), :], x3[:, ds(4 * d, 4), :])

        # bf16 copy of x for transposes
        x_bf = consts.tile([P, NT, C], BF16)
        for d in range(4):
            nc.any.tensor_copy(x_bf[:, ds(8 * d, 8), :],
                               x_sb[:, ds(8 * d, 8), :])

        # ACT exp bias: exp(s'/C1 + bias) = exp(s - ln 8)
        ebias = consts.tile([P, 1], F32)
        nc.vector.memset(ebias[:], float(-SCORE_OFF / C1 - np.log(8.0)))

        # ---- xT_aug [128, N] bf16: rows 0..63 = x^T, row 64 = ones, 65.. = 0 ----
        xt_sb = consts.tile([128, N], BF16)
        nc.vector.memset(xt_sb[C:, :], 0.0)
        nc.vector.memset(xt_sb[C: C + 1, :], 1.0)

        with tc.tile_pool(name="pro_ps", bufs=2, space="PSUM") as pro_ps:
            # transpose x tiles in groups of 4 -> one PSUM bank per group
            for grp in range(NT // 4):
                pt = pro_ps.tile([C, 512], BF16, tag="xtr")
                for j in range(4):
                    t = 4 * grp + j
                    nc.tensor.transpose(pt[:, ts(j, P)], x_bf[:, t, :], id_sb[:])
                nc.any.tensor_copy(xt_sb[:C, ds(grp * 512, 512)], pt[:])

            # f/g projections (f scaled by C1), band-replicated
            f_sb = consts.tile([128, N], BF16)
            g_sb = consts.tile([128, N], BF16)
            for chunk in range(N // 512):
                pf = pro_ps.tile([128, 512], F32, tag="fg")
                nc.tensor.matmul(pf[:], wf_sb[:, :], xt_sb[:, ts(chunk, 512)],
                                 start=True, stop=True)
                nc.any.tensor_copy(f_sb[:, ts(chunk, 512)], pf[:])
            for chunk in range(N // 512):
                pg = pro_ps.tile([128, 512], F32, tag="fg")
                nc.tensor.matmul(pg[:], wg_sb[:, :], xt_sb[:, ts(chunk, 512)],
                                 start=True, stop=True)
                nc.any.tensor_copy(g_sb[:, ts(chunk, 512)], pg[:])

            # h_aug tiles in fp8: h_sb[:, m, :] = [128*gamma*h | 128 | 0pad];
            # adjacent m-tiles form the [128, 2, 128] DoubleRow weight pairs
            h_sb = consts.tile([P, NT, 128], FP8)
            for grp in range(NT // 4):
                ph = pro_ps.tile([128, 512], F32, tag="h")
                for j in range(4):
                    m = 4 * grp + j
                    nc.tensor.matmul(ph[:, ts(j, P)], xt_sb[:, ts(m, P)],
                                     wh_sb[:], start=True, stop=True)
                nc.any.tensor_copy(h_sb[:, ds(4 * grp, 4), :], ph[:])

        # ---- main: scores -> exp -> ctxT accumulate; epilogue, per n-quarter ----
        QW = 1024                      # quarter width
        NQ = N // QW                   # 4
        QT = QW // P                   # n-tiles per quarter
        with tc.tile_pool(name="ps_s", bufs=3, space="PSUM") as ps_s, \
             tc.tile_pool(name="ps_ctx", bufs=1, space="PSUM") as ps_ctx:
            # exp engine assignment: ACT chunk ~997ns vs DVE ~1192ns (+ DVE's
            # prologue/epilogue load) -> give ACT ~81 of 128 chunks, spread
            # evenly (Bresenham).
            N_CHUNKS = NQ * NT
            DVE_SHARE = 58
            use_dve = [((i * DVE_SHARE) % N_CHUNKS) < DVE_SHARE
                       for i in range(N_CHUNKS)]

            for q in range(NQ):
                ctx_ps = ps_ctx.tile([128, QW], F32, tag="ctx")
                for mg in range(NT // 2):
                    # two m-tiles' score matmuls packed in 32-row PE tiles
                    sp = [ps_s.tile([128, QW], F32, tag="s", name=f"s{b}")
                          for b in range(2)]
                    for b in range(2):
                        m = 2 * mg + b
                        for j in range(QW // 512):
                            nc.tensor.matmul(
                                sp[b][:, ts(j, 512)],
                                g_sb[ds(32 * b, 32), ts(m, P)],
                                f_sb[ds(32 * b, 32),
                                     ds(q * QW + j * 512, 512)],
                                start=True, stop=True,
                                tile_position=(32 * b, 0))
                    e_pair = exp_pool.tile([128, 2, QW], FP8, tag="e")
                    for b in range(2):
                        m = 2 * mg + b
                        if use_dve[q * NT + m]:
                            _dve_exp(nc, work, e_pair, b, sp[b], QW)
                        else:
                            nc.scalar.activation(
                                e_pair[:, b, :], sp[b][:],
                                mybir.ActivationFunctionType.Exp,
                                bias=ebias[:], scale=float(1.0 / C1))
                    for j in range(QW // 512):
                        nc.tensor.matmul(
                            ctx_ps[:, ts(j, 512)],
                            h_sb[:, ds(2 * mg, 2), :],
                            e_pair[:, :, ts(j, 512)],
                            perf_mode=mybir.MatmulPerfMode.DoubleRow,
                            start=(mg == 0), stop=(mg == NT // 2 - 1))

                # epilogue for this quarter: DMA-transpose ctxT back to [n, c]
                ctxt_sb = work.tile([128, QW], BF16, tag="ctxt")
                nc.any.tensor_copy(ctxt_sb[:], ctx_ps[:])
                o_tr = work.tile([128, QT, P], BF16, tag="otr")
                for t in range(QT):
                    nc.sync.dma_start_transpose(o_tr[:, t, :],
                                                ctxt_sb[:, ts(t, P)])
                for t in range(QT):
                    blk = o_tr[:, t, :]
                    rden = work.tile([P, 1], F32, tag="rden")
                    nc.vector.reciprocal(rden[:], blk[:, C: C + 1])
                    tmp = work.tile([P, C], F32, tag="tmp")
                    nc.vector.tensor_scalar_mul(tmp[:], blk[:, :C], rden[:])
                    osb = work.tile([P, C], F32, tag="osb")
                    nc.gpsimd.tensor_add(osb[:], tmp[:],
                                         x_sb[:, q * QT + t, :])
                    nc.sync.dma_start(
                        out_d[ds((q * QT + t) * P, P), :], osb[:])


def _dve_exp(nc, work, e_pair, b, s_ps, ncols):
    """fp8e4m3 bit-trick exp on the DVE: i8 = round(max(s'/16, 0))
    reinterpreted as e4m3 ~= exp(s)/8.  s' = C1*s + SCORE_OFF (from the
    weights), so s'/16 = 8*log2(e)*s + 31.5 -- the e4m3 bit pattern of
    exp(s)/8; ultra-negative scores clamp to +0."""
    i8_view = e_pair.bitcast(mybir.dt.int8)
    nc.vector.tensor_scalar(i8_view[:, b, :ncols], s_ps[:, :ncols],
                            1.0 / 16.0, 0.0,
                            mybir.AluOpType.mult, mybir.AluOpType.max)


_CACHE = {}


def _get_nc():
    if "nc" not in _CACHE:
        _CACHE["nc"] = build_bass()
    return _CACHE["nc"]


def kernel(x, Wf, bf, Wg, bg, Wh, bh, gamma):
    x = np.asarray(x, np.float32)
    B = x.shape[0]
    assert x.shape == (B, 64, 64, 64) and B == 8

    w = prepare_weights(Wf, bf, Wg, bg, Wh, bh, gamma)
    nc = _get_nc()
    in_maps = [
        {"x": np.ascontiguousarray(x[i].reshape(N, C)), **w}
        for i in range(B)
    ]
    res = run_bass_kernel_spmd(nc, in_maps, core_ids=list(range(8)))
    out = np.stack([np.asarray(res.results[i]["out"]).reshape(64, 64, 64)
                    for i in range(B)])
    return out.astype(np.float32)
